# revision 39
# baseline (speedup 1.0000x reference)
"""Trainium2 Bass kernel for nn_Attention (B=4, S=1024, DIM=1024, H=16, Dh=64).

Sharding: 8 cores = 4 batches x 2 head-groups (8 heads / 512 inner channels
each).  Each core computes q/k/v projections for its head shard, RoPE,
attention, and a partial output projection (its rows of Wo); the host sums
the two head-group partials per batch and concatenates batches.

v3 (vs baseline): the PE column-pipe is the bottleneck (cost = sum of rhs
columns over all matmuls, ~0.42ns/col regardless of dtype -- fp8 DoubleRow
measured no faster).  Structural cuts only:
  - rowsum matmuls (128 x 512 cols) eliminated: V is augmented with a ones
    column (M=65) so P@V_aug yields the softmax denominators for free.
  - bias matmuls eliminated: V bias added on the DVE spill; output bias
    added from a host-precomputed masked-bias tensor (mbo) on the spill.
  - attention runs as per-(row-tile, head) passes so the two live PV
    accumulators fit PSUM next to double-buffered score tiles.
  - out-mask folded into the reciprocal (recq = mask/rowsum).
  - inputs DMA'd in kt-chunks with the first projections interleaved at
    chunk granularity; f16 output DMA.
"""

import numpy as np

B, S, DIM, HEADS, HEAD_DIM = 4, 1024, 1024, 16, 64
INNER = HEADS * HEAD_DIM
HG = 2                      # head groups (tensor-parallel shards)
DSH = INNER // HG           # 512 inner channels per core
HSH = HEADS // HG           # 8 heads per core
NCORES = B * HG
KT = DIM // 128             # 8 contraction tiles
MT = DSH // 128             # 4 row tiles for Q^T/K^T
ST = S // 128               # 8 seq tiles
MASK_NEG = -80.0
WS = 1.0      # weight scale (1 = plain f16 weights)

_CACHE = {}


def _build():
    import concourse.tile as tile
    from concourse import bacc, mybir

    f32 = mybir.dt.float32
    f16 = mybir.dt.float16
    f8 = mybir.dt.float8e4
    AF = mybir.ActivationFunctionType
    OP = mybir.AluOpType

    nc = bacc.Bacc("TRN2", target_bir_lowering=False, debug=False)

    xT_d = nc.dram_tensor("xT", [128, KT, S], f16, kind="ExternalInput")
    wq_d = nc.dram_tensor("wq", [128, MT, KT, 128], f16, kind="ExternalInput")
    wk_d = nc.dram_tensor("wk", [128, MT, KT, 128], f16, kind="ExternalInput")
    wv_d = nc.dram_tensor("wv", [128, KT, DSH], f16, kind="ExternalInput")
    wo_d = nc.dram_tensor("wo", [128, MT, DIM], f16, kind="ExternalInput")
    bq_d = nc.dram_tensor("bq", [128, MT], f32, kind="ExternalInput")
    bk_d = nc.dram_tensor("bk", [128, MT], f32, kind="ExternalInput")
    bvbc_d = nc.dram_tensor("bvbc", [128, DSH], f16, kind="ExternalInput")
    bobc_d = nc.dram_tensor("bobc", [128, DIM], f16, kind="ExternalInput")
    m01c_d = nc.dram_tensor("m01c", [128, ST], f32, kind="ExternalInput")
    cos_d = nc.dram_tensor("cos2", [128, S], f16, kind="ExternalInput")
    sin_d = nc.dram_tensor("sin2", [128, S], f16, kind="ExternalInput")
    prt_d = nc.dram_tensor("prt", [128, 128], f16, kind="ExternalInput")
    maskb_d = nc.dram_tensor("maskb", [128, ST], f32, kind="ExternalInput")
    mrowt_d = nc.dram_tensor("mrowt", [97, 512], f16, kind="ExternalInput")
    ones4_d = nc.dram_tensor("ones4", [97, HEAD_DIM], f16,
                             kind="ExternalInput")
    ident_d = nc.dram_tensor("ident", [128, 128], f16, kind="ExternalInput")
    out_d = nc.dram_tensor("out", [S, DIM], f16, kind="ExternalOutput")
    recd = nc.dram_tensor("recd", [97, MT, 512], f16)

    with tile.TileContext(nc) as tc, \
         tc.tile_pool(name="persist", bufs=1) as persist, \
         tc.tile_pool(name="p1sb", bufs=3) as p1sb, \
         tc.tile_pool(name="p2r", bufs=2) as p2r:

        xT = persist.tile([128, KT, S], f16)
        qT = persist.tile([128, MT, S], f16)
        kT = persist.tile([128, MT, S], f16)
        vv = persist.tile([128, KT, HSH, 65], f16)   # 65th col = 1.0 (rowsum)
        attU = persist.tile([128, MT, S], f16)
        wo = persist.tile([128, MT, DIM], f16)
        mbo = persist.tile([128, ST, DIM], f16)
        bobc = persist.tile([128, DIM], f16)
        m01c = persist.tile([128, ST], f32)
        maskb = persist.tile([128, ST], f32)
        mrowt = persist.tile([97, 512], f16)
        ones4 = persist.tile([97, HEAD_DIM], f16)
        rssum = persist.tile([97, MT, 512], f32)
        recf = persist.tile([97, MT, 512], f32)
        recq = persist.tile([97, MT, 512], f16)
        bq = persist.tile([128, MT], f32)
        bk = persist.tile([128, MT], f32)
        bvbc = persist.tile([128, DSH], f16)
        part01 = persist.tile([128, ST, DIM], f16)
        ident = persist.tile([128, 128], f16)
        wq = persist.tile([128, MT, KT, 128], f16)
        wk = persist.tile([128, MT, KT, 128], f16)

        wv = persist.tile([128, KT, DSH], f16)

        with tc.tile_pool(name="w1", bufs=1) as w1:
            cos2 = w1.tile([128, S], f16)
            sin2 = w1.tile([128, S], f16)
            prt = w1.tile([128, 128], f16)

            # DMA priority: the prologue's critical bytes are x, the mt=0
            # slices of Wq/Wk (256KB each) and wv (kt-streamed); the mt1-3
            # weight bulk is zipper material needed ~20us later, and
            # wo/mbo are phase-3 only.
            for p in range(4):
                ksl = slice(2 * p, 2 * p + 2)
                nc.sync.dma_start(out=xT[:, ksl, :], in_=xT_d.ap()[:, ksl, :])
            nc.scalar.dma_start(out=wk[:, 0], in_=wk_d.ap()[:, 0])
            nc.gpsimd.dma_start(out=wq[:, 0], in_=wq_d.ap()[:, 0])
            for p in range(4):
                ksl = slice(2 * p, 2 * p + 2)
                nc.scalar.dma_start(out=wv[:, ksl], in_=wv_d.ap()[:, ksl])
            for t, d in [(bobc, bobc_d), (m01c, m01c_d), (bq, bq_d),
                         (bk, bk_d), (cos2, cos_d),
                         (sin2, sin_d), (prt, prt_d), (maskb, maskb_d),
                         (bvbc, bvbc_d), (ones4, ones4_d), (mrowt, mrowt_d),
                         (ident, ident_d)]:
                nc.gpsimd.dma_start(out=t[:], in_=d.ap())
            nc.scalar.dma_start(out=wk[:, 1:4], in_=wk_d.ap()[:, 1:4])
            nc.gpsimd.dma_start(out=wq[:, 1:4], in_=wq_d.ap()[:, 1:4])
            nc.scalar.dma_start(out=wo[:], in_=wo_d.ap())

            # masked output bias built on ACT (idle all prologue) instead
            # of a 2MB DMA: mbo[:,qt,:] = bobc * mask-column
            for qt in range(ST):
                nc.scalar.activation(mbo[:, qt, :], bobc[:], AF.Copy,
                                     scale=m01c[:, qt:qt + 1])

            # V_aug ones column + Exp table preload while DMAs stream
            nc.vector.memset(vv[:, :, :, 64:65], 1.0)
            tiny = w1.tile([128, 1], f32)
            tiny2 = w1.tile([128, 1], f16)
            nc.vector.memset(tiny[:], 0.0)
            nc.scalar.activation(tiny2[:], tiny[:], AF.Exp)

            def rope_apply(dst, b, c2, ps, pppool):
                # RoPE on the first 64 flat channels only (rows 64-127 and
                # the hg=1 core get identity via cos=1/sin=0 from the host).
                sl = slice(c2 * 512, (c2 + 1) * 512)
                sinp = p1sb.tile([128, 512], f16, tag="sinp", name="sinp")
                nc.vector.scalar_tensor_tensor(
                    sinp[:], ps[:], b[:, 0:1],
                    sin2[:, sl], op0=OP.add, op1=OP.mult)
                cosp = p1sb.tile([128, 512], f32, tag="cosp", name="cosp")
                nc.vector.scalar_tensor_tensor(
                    cosp[:], ps[:], b[:, 0:1],
                    cos2[:, sl], op0=OP.add, op1=OP.mult)
                pp = pppool.tile([128, 512], f32, tag="pp", name="pp")
                nc.tensor.matmul(out=pp[:], lhsT=prt[:], rhs=sinp[:],
                                 start=True, stop=True)
                nc.vector.tensor_tensor(
                    dst[:, 0, sl], cosp[:], pp[:], op=OP.add)

            def v_spill(st, ps):
                # V bias-add fused into the PSUM->SBUF move; writes cols
                # 0:64 of the augmented vv rows (col 64 stays 1.0)
                nc.vector.tensor_tensor(
                    vv[:, st, :, 0:64],
                    ps[:].rearrange("p (h d) -> p h d", h=HSH),
                    bvbc[:].rearrange("p (h d) -> p h d", h=HSH),
                    op=OP.add)

            # ---- prologue: K0/Q0/V0/V1 kt-major (DMA-paced), RoPE, V ----
            with tc.tile_pool(name="vps", bufs=6, space="PSUM") as vps, \
                 tc.tile_pool(name="p1pp", bufs=2, space="PSUM") as p1pp:
                kq_ps = []
                for dst, w, b in ((kT, wk, bk), (qT, wq, bq)):
                    for c2 in range(2):
                        ps = vps.tile([128, 512], f32, tag="vps", name="ps")
                        kq_ps.append((dst, w, b, c2, ps))
                v_ps = {st: vps.tile([128, DSH], f32, tag="vps", name="ps")
                        for st in range(2)}
                for kt in range(KT):   # kt-major: start on the first chunk
                    for dst, w, b, c2, ps in kq_ps:
                        nc.tensor.matmul(
                            out=ps[:], lhsT=w[:, 0, kt, :],
                            rhs=xT[:, kt, c2 * 512:(c2 + 1) * 512],
                            start=(kt == 0), stop=(kt == KT - 1))
                    for st in range(2):
                        nc.tensor.matmul(
                            out=v_ps[st],
                            lhsT=xT[:, kt, st * 128:(st + 1) * 128],
                            rhs=wv[:, kt, :],
                            start=(kt == 0), stop=(kt == KT - 1))
                for dst, w, b, c2, ps in kq_ps:
                    rope_apply(dst, b, c2, ps, p1pp)
                for st in range(2):
                    v_spill(st, v_ps[st])
                for st in range(2, ST - 2):
                    ps = vps.tile([128, DSH], f32, tag="vps", name="ps")
                    for kt in range(KT):
                        nc.tensor.matmul(
                            out=ps[:],
                            lhsT=xT[:, kt, st * 128:(st + 1) * 128],
                            rhs=wv[:, kt, :],
                            start=(kt == 0), stop=(kt == KT - 1))
                    v_spill(st, ps)


        # ---- attention: per-(row-tile, head) passes --------------------
        # P@V runs one kt slot behind the exps so it never stalls the
        # in-order PE queue waiting for the current exp; score tiles
        # double-buffer by kt parity.
        with tc.tile_pool(name="p1ps", bufs=2, space="PSUM") as p1ps:
            with tc.tile_pool(name="p2sc", bufs=1, space="PSUM") as p2sc, \
                 tc.tile_pool(name="p2at", bufs=1, space="PSUM") as p2at, \
                 tc.tile_pool(name="p2pt", bufs=3) as p2pt:

                # zippered projections, one matmul per next(): K/Q for
                # row-tiles 1-3, then the mt0+mt1 half of the output
                # projection (fills the last two passes).
                def proj_units():
                    # V st6-7 first (needed by pass-0 slots 6-7)
                    for st in range(ST - 2, ST):
                        ps = p1ps.tile([128, DSH], f32, tag="ps",
                                       name="ps")
                        for kt in range(KT):
                            nc.tensor.matmul(
                                out=ps[:],
                                lhsT=xT[:, kt, st * 128:(st + 1) * 128],
                                rhs=wv[:, kt, :],
                                start=(kt == 0), stop=(kt == KT - 1))
                            if kt == KT - 1:
                                v_spill(st, ps)
                            yield
                    for mt in range(1, MT):
                        for dst, w, b in ((kT, wk, bk), (qT, wq, bq)):
                            for c2 in range(2):
                                qsl = slice(c2 * 512, (c2 + 1) * 512)
                                ps = p1ps.tile([128, 512], f32, tag="ps",
                                               name="ps")
                                for kt in range(KT):
                                    nc.tensor.matmul(
                                        out=ps[:], lhsT=w[:, mt, kt, :],
                                        rhs=xT[:, kt, qsl],
                                        start=(kt == 0), stop=(kt == KT - 1))
                                    if kt == KT - 1:
                                        nc.vector.tensor_scalar(
                                            dst[:, mt, qsl], ps[:],
                                            1.0 / WS, b[:, mt:mt + 1],
                                            op0=OP.mult, op1=OP.add)
                                    yield
                    for qt in range(ST):
                        for c2 in range(2):
                            nsl = slice(c2 * 512, (c2 + 1) * 512)
                            ps = p1ps.tile([128, 512], f32, tag="ps",
                                           name="ps")
                            for mt in range(2):
                                nc.tensor.matmul(
                                    out=ps[:],
                                    lhsT=attU[:, mt,
                                              qt * 128:(qt + 1) * 128],
                                    rhs=wo[:, mt, nsl],
                                    start=(mt == 0), stop=(mt == 1))
                                if mt == 1:
                                    nc.vector.tensor_tensor(
                                        part01[:, qt, nsl], ps[:],
                                        mbo[:, qt, nsl], op=OP.add)
                                yield
                    while True:
                        yield

                def normalize(mt):
                    # DRAM bounce partition-broadcasts each head's masked
                    # 1/rowsum row; the multiply runs on idle GpSimd.
                    nc.gpsimd.dma_start(out=recd.ap()[:, mt, :],
                                        in_=recq[:, mt, :])
                    for hh in range(2):
                        ph = hh * 64
                        rb = p2r.tile([128, 2, 512], f32, tag="rb",
                                      name="rb")
                        nc.gpsimd.dma_start(
                            out=rb[ph:ph + 64],
                            in_=recd.ap()[64 * hh:64 * hh + 33:32,
                                          mt, :]
                            .partition_broadcast(HEAD_DIM))
                        nc.gpsimd.tensor_tensor(
                            attU[ph:ph + 64, mt, :],
                            attU[ph:ph + 64, mt, :],
                            rb[ph:ph + 64].rearrange("p a b -> p (a b)"),
                            op=OP.mult)

                gen = proj_units()
                # zipper matmuls per kt slot (96 K/Q + 32 out-proj)
                ZC = (3, 3, 2, 2, 2, 2, 2, 2)

                def emit_scores(mt, hh, kt):
                    t = p2sc.tile([128, S], f32, tag=f"sc{kt % 2}",
                                  name=f"sc{kt % 2}")
                    ph = hh * 64
                    for c2 in range(2):
                        nc.tensor.matmul(
                            out=t[:, c2 * 512:c2 * 512 + 512],
                            lhsT=kT[ph:ph + 64, mt,
                                    kt * 128:(kt + 1) * 128],
                            rhs=qT[ph:ph + 64, mt,
                                   c2 * 512:(c2 + 1) * 512],
                            start=True, stop=True, tile_position=(ph, 0))
                    return t

                def emit_pv(at, hv, kt, pt):
                    for c2 in range(2):
                        nc.tensor.matmul(
                            out=at[c2][:], lhsT=vv[:, kt, hv, :],
                            rhs=pt[:, c2 * 512:(c2 + 1) * 512],
                            start=(kt == 0), stop=(kt == ST - 1))

                for pi in range(MT * 2):
                    mt, hh = pi // 2, pi % 2
                    hv = mt * 2 + hh
                    cnt = ZC[pi]
                    at = {c2: p2at.tile([65, 512], f32, name=f"at{c2}",
                                        tag=f"at{c2}")
                          for c2 in range(2)}
                    sch = emit_scores(mt, hh, 0)
                    pt_prev = None
                    for kt in range(ST):
                        pt = p2pt.tile([128, S], f16, tag="pt", name="pt")
                        nc.scalar.activation(
                            pt[:], sch[:], AF.Exp,
                            bias=maskb[:, kt:kt + 1], scale=0.125)
                        # PE order: lagged PV (no deps) and the next score
                        # tile first -- the following exp must not wait --
                        # then zipper matmuls fill the rest of the slot
                        if kt > 0:
                            emit_pv(at, hv, kt - 1, pt_prev)
                        if kt < ST - 1:
                            sch = emit_scores(mt, hh, kt + 1)
                        for _ in range(cnt):
                            next(gen)
                        pt_prev = pt
                        if kt == 2 and pi > 1 and hh == 0:
                            normalize(mt - 1)
                    emit_pv(at, hv, ST - 1, pt_prev)
                    # pass epilogue: rowsum row first (it gates the
                    # normalize chain), attn spill after.  The last pass's
                    # rowsums ride on ACT (idle once its exps retire).
                    for c2 in range(2):
                        r = 32 * (2 * hh + c2)
                        if pi == MT * 2 - 1:
                            nc.scalar.activation(rssum[r:r + 1, mt, :],
                                                 at[c2][64:65, :], AF.Copy)
                        else:
                            nc.vector.tensor_copy(rssum[r:r + 1, mt, :],
                                                  at[c2][64:65, :])
                    for c2 in range(2):
                        qsl = slice(c2 * 512, (c2 + 1) * 512)
                        nc.vector.tensor_copy(attU[hh * 64:hh * 64 + 64,
                                                   mt, qsl],
                                              at[c2][0:64, :])
                    if hh == 1:
                        # row-tile complete: masked reciprocal (junk rows
                        # of rssum produce junk never read downstream)
                        nc.vector.reciprocal_approx_fast(
                            recf[:, mt, :], rssum[:, mt, :])
                        nc.vector.tensor_tensor(
                            recq[:, mt, :], recf[:, mt, :], mrowt[:],
                            op=OP.mult)

            # ---- phase 3 (p2 pools closed; p1ps still open) ------------
            with tc.tile_pool(name="p3ps", bufs=6, space="PSUM") as p3ps, \
                 tc.tile_pool(name="p3sb", bufs=3) as p3sb:
                pre = {}
                for qt in range(3):
                    # pre-start mt=2 steps BEFORE the mt3 normalize ops so
                    # the PE stays busy through the reciprocal chain
                    for c2 in range(2):
                        nsl = slice(c2 * 512, (c2 + 1) * 512)
                        ps = p3ps.tile([128, 512], f32, tag="ps3",
                                       name="ps3")
                        nc.tensor.matmul(
                            out=ps[:],
                            lhsT=attU[:, 2, qt * 128:(qt + 1) * 128],
                            rhs=wo[:, 2, nsl], start=True, stop=False)
                        pre[qt, c2] = ps
                # last row-tile: normalize via PE broadcast (the DRAM
                # bounce's DMA latency would sit fully exposed here)
                for c2 in range(2):
                    qsl = slice(c2 * 512, (c2 + 1) * 512)
                    for hh in range(2):
                        ph = hh * 64
                        r = 32 * (2 * hh + c2)
                        rbps = p1ps.tile([HEAD_DIM, 512], f32, tag="ps",
                                         name="rbps")
                        nc.tensor.matmul(
                            out=rbps[:], lhsT=ones4[r:r + 1, :],
                            rhs=recq[r:r + 1, MT - 1, :],
                            start=True, stop=True, tile_position=(r, 0))
                        nc.vector.tensor_tensor(
                            attU[ph:ph + 64, MT - 1, qsl],
                            attU[ph:ph + 64, MT - 1, qsl],
                            rbps[:], op=OP.mult)
                for qt in range(ST):
                    ob = p3sb.tile([128, DIM], f16, tag="ob")
                    for c2 in range(2):
                        nsl = slice(c2 * 512, (c2 + 1) * 512)
                        if (qt, c2) in pre:
                            ps = pre[qt, c2]
                        else:
                            ps = p3ps.tile([128, 512], f32, tag="ps3",
                                           name="ps3")
                            nc.tensor.matmul(
                                out=ps[:],
                                lhsT=attU[:, 2, qt * 128:(qt + 1) * 128],
                                rhs=wo[:, 2, nsl], start=True, stop=False)
                        if c2 == 1:
                            # fold part01 into PSUM via an identity matmul
                            # so the spill is a plain ACT copy (DVE and ACT
                            # split the phase-3 spill load)
                            nc.tensor.matmul(
                                out=ps[:], lhsT=ident[:],
                                rhs=part01[:, qt, nsl],
                                start=False, stop=False)
                        nc.tensor.matmul(
                            out=ps[:],
                            lhsT=attU[:, 3, qt * 128:(qt + 1) * 128],
                            rhs=wo[:, 3, nsl], start=False, stop=True)
                        if c2 == 1:
                            nc.scalar.activation(ob[:, nsl], ps[:],
                                                 AF.Copy)
                            nc.gpsimd.dma_start(
                                out=out_d.ap()[qt * 128:(qt + 1) * 128,
                                               nsl],
                                in_=ob[:, nsl])
                        else:
                            nc.vector.tensor_tensor(
                                ob[:, nsl], ps[:], part01[:, qt, nsl],
                                op=OP.add)
                            nc.sync.dma_start(
                                out=out_d.ap()[qt * 128:(qt + 1) * 128,
                                               nsl],
                                in_=ob[:, nsl])

    nc.compile()
    return nc


def _get_nc():
    if "nc" not in _CACHE:
        _CACHE["nc"] = _build()
    return _CACHE["nc"]


def _prep_inputs(x, mask, freqs, Wq, bq, Wk, bk, Wv, bv, Wo, bo):
    f = np.asarray(freqs, np.float32)[0]              # [S, HEAD_DIM]
    # reference rotates only the first rot_dim=64 channels of the FLAT
    # inner dim -> rows 0-63 of row-tile 0 on the hg=0 core; everything
    # else is identity (cos=1, sin=0).
    cos2 = np.full((128, S), 1.0 / WS, np.float32)
    sin2 = np.zeros((128, S), np.float32)
    cos2[0:HEAD_DIM] = np.cos(f.T) / WS
    sin2[0:HEAD_DIM] = np.sin(f.T) / WS
    cos2 = cos2.astype(np.float16)
    sin2 = sin2.astype(np.float16)
    ident = np.full((128, S), 1.0 / WS, np.float16)
    identz = np.zeros((128, S), np.float16)

    prt = np.zeros((128, 128), np.float16)            # P_rot^T
    i = np.arange(0, 128, 2)
    prt[i + 1, i] = -1.0                              # P_rot[2i, 2i+1] = -1
    prt[i, i + 1] = 1.0                               # P_rot[2i+1, 2i] = +1

    ones4 = np.ones((97, HEAD_DIM), np.float16)
    ident128 = np.eye(128, dtype=np.float16)

    def lhsT_w(w):       # [DIM, DSH] -> mt-major lhsT tiles
        return np.ascontiguousarray(
            (w * WS).reshape(KT, 128, MT, 128)
            .transpose(1, 2, 0, 3)).astype(np.float16)

    def col(b):          # [DSH] -> [128, MT]; col 0 pre-scaled x16 (RoPE)
        c = np.ascontiguousarray(b.reshape(MT, 128).T.astype(np.float32))
        c[:, 0] *= WS
        return c

    bo_half = np.asarray(bo, np.float32) * 0.5
    bobc = np.broadcast_to(bo_half, (128, DIM)).astype(np.float16).copy()

    in_maps = []
    for b in range(B):
        xT = np.ascontiguousarray(
            np.asarray(x[b], np.float32).T.reshape(KT, 128, S)
            .transpose(1, 0, 2)).astype(np.float16)
        m = np.asarray(mask[b])
        m01 = m.astype(np.float32)
        maskb = np.ascontiguousarray(
            np.where(m, 0.0, MASK_NEG).astype(np.float32).reshape(ST, 128).T)
        mrowt = np.zeros((97, 512), np.float16)
        for hh in range(2):
            for c2 in range(2):
                mrowt[32 * (2 * hh + c2)] = m01[c2 * 512:(c2 + 1) * 512]
        m01c = np.ascontiguousarray(
            m01.reshape(ST, 128).T).astype(np.float32)
        for hg in range(HG):
            dsl = slice(hg * DSH, (hg + 1) * DSH)
            in_maps.append({
                "xT": xT,
                "wq": lhsT_w(np.asarray(Wq, np.float32)[:, dsl]),
                "wk": lhsT_w(np.asarray(Wk, np.float32)[:, dsl]),
                "wv": np.ascontiguousarray(
                    np.asarray(Wv, np.float32)[:, dsl]
                    .reshape(KT, 128, DSH).transpose(1, 0, 2)).astype(np.float16),
                "wo": np.ascontiguousarray(
                    np.asarray(Wo, np.float32)[dsl, :]
                    .reshape(MT, 128, DIM).transpose(1, 0, 2)).astype(np.float16),
                "bq": col(np.asarray(bq, np.float32)[dsl]),
                "bk": col(np.asarray(bk, np.float32)[dsl]),
                "bvbc": np.broadcast_to(
                    np.asarray(bv, np.float32)[dsl], (128, DSH))
                    .astype(np.float16).copy(),
                "bobc": bobc,
                "m01c": m01c,
                "cos2": cos2 if hg == 0 else ident,
                "sin2": sin2 if hg == 0 else identz,
                "prt": prt,
                "maskb": maskb,
                "mrowt": mrowt,
                "ones4": ones4,
                "ident": ident128,
            })
    return in_maps


def run(trace=False, **inputs):
    from concourse import bass_utils
    if trace:
        _install_ntff_hook()
    nc = _get_nc()
    in_maps = _prep_inputs(**inputs)
    res = bass_utils.run_bass_kernel_spmd(
        nc, in_maps, core_ids=list(range(NCORES)), trace=trace)
    out = np.empty((B, S, DIM), np.float32)
    for b in range(B):
        out[b] = (res.results[2 * b]["out"].astype(np.float32)
                  + res.results[2 * b + 1]["out"].astype(np.float32))
    return out, res


def kernel(**inputs):
    out, _ = run(trace=False, **inputs)
    return out


def _install_ntff_hook():
    """Register the axon NTFF profiling hook missing from the antenv stub."""
    import sys, types
    try:
        import antenv.axon_hooks  # noqa: F401
        return
    except ImportError:
        pass
    from trn_agent_boot.trn_boot import _ntff_profile_via_ctypes
    hook = _ntff_profile_via_ctypes('/opt/axon/libaxon_pjrt.so')
    mod = types.ModuleType('antenv.axon_hooks')
    mod.get_axon_ntff_profile_hook = lambda: hook
    mod.set_axon_ntff_profile_hook = lambda h: None
    sys.modules['antenv.axon_hooks'] = mod
